# revision 1
# baseline (speedup 1.0000x reference)
"""Trainium2 Bass kernel for the Actor MLP scorer (gnn_message_passing).

Computation (see reference):
    node_e  = node_embeddings[action_nodes]          # [A, 128] gather
    feats   = [node_e | region_embeddings[action_regions] | const_tail]   # [A, 1427]
    h1..h3  = relu MLP (256 wide), logits = h3 @ W4 + b4                  # [A]
    probs   = softmax(logits) over ALL actions

Strategy (8 NeuronCores, data-parallel over actions):
  - Shard A=100000 actions as 12500/core.  Per core, actions are sorted by
    node-id bucket (< 32768 vs >= 32768) so the node-embedding gather can use
    the int16-indexed DMA-gather ucode with two base-offset views of a bf16
    copy of the table; transpose mode deposits embeddings directly in
    [dim, action] layout (no on-chip transposes).  Groups are padded to the
    static capacities C0/C1 (~7 sigma for uniform node ids); a mask input
    removes pad slots from the softmax.  Outputs are un-permuted on host.
  - Layer 1 is decomposed: feats @ W1 = node_e @ W1[:128]
        + onehot(region) @ (region_embeddings @ W1[128:256])
        + tail @ W1[256:]  (constant -> folded into the relu bias).
    The constant tail/region projections are computed on-device in a
    fp32 prologue.
  - Activations stay transposed ([feature, action]); matmuls are bf16 with
    fp32 PSUM accumulation; relu+bias evictions split across ScalarE/VectorE.
  - Softmax: per-core sum(exp(logit - 4)), one [1,1] AllReduce over the 8
    cores, then probs = exp * (1/S) on-core.
"""

import sys

for _p in ("/opt/trn_rl_repo",):
    if _p not in sys.path:
        sys.path.insert(0, _p)

import numpy as np
import ml_dtypes
from concourse import bass, bacc, mybir, tile
from concourse import bass_utils
from concourse.masks import make_identity


# ---------------------------------------------------------------- constants
N_CORES = 8
A_FULL = 100000
N_NODES = 50000
N_REGIONS = 8
D = 128
H = 256
G = 147
IN_DIM = 2 * D + N_REGIONS * D + G          # 1427
TAIL_LEN = N_REGIONS * D + G                # 1171
TAIL_KT = 10                                # ceil(1171/128)
F32 = mybir.dt.float32
BF16 = mybir.dt.bfloat16
I16 = mybir.dt.int16

A_PC = A_FULL // N_CORES                    # 12500
SPLIT = 32768                               # int16 index range boundary
C0 = 8704                                   # capacity, node id < 32768 (17*512)
C1 = 4608                                   # capacity, node id >= 32768 (9*512)
A_PAD = C0 + C1                             # 13312 = 26*512 = 104*128
N_CHUNKS = A_PAD // 128                     # 104
ATILE = 512
N_AT = A_PAD // ATILE                       # 26
GCHUNK = 1024                               # idxs per dma_gather call

EXP_SHIFT = -4.0


def _gather_chunks(total):
    out, off = [], 0
    while off < total:
        n = min(GCHUNK, total - off)
        out.append((off, n))
        off += n
    return out


def build_graph():
    nc = bacc.Bacc("TRN2", target_bir_lowering=False, debug=False,
                   num_devices=N_CORES, num_swdge_queues=4)

    # ---- I/O --------------------------------------------------------------
    node_emb = nc.dram_tensor("node_emb", [N_NODES, D], BF16, kind="ExternalInput")
    w1 = nc.dram_tensor("w1", [IN_DIM, H], F32, kind="ExternalInput")
    w2 = nc.dram_tensor("w2", [H, H], F32, kind="ExternalInput")
    w3 = nc.dram_tensor("w3", [H, H], F32, kind="ExternalInput")
    # small per-core constants packed into one tensor (one DMA):
    # cols 0:2 b1c | 2:4 b2c | 4:6 b3c | 6:8 w4c | 8:16 regT | 16:26 tailc
    # | 26:130 mask | [0,130] b4
    packed = nc.dram_tensor("packed", [128, 131], F32, kind="ExternalInput")
    idx0 = nc.dram_tensor("idx0", [128, C0 // 16], I16, kind="ExternalInput")
    idx1 = nc.dram_tensor("idx1", [128, C1 // 16], I16, kind="ExternalInput")
    onehot = nc.dram_tensor("onehot", [N_REGIONS, A_PAD], BF16, kind="ExternalInput")

    out_logits = nc.dram_tensor("out_logits", [1, A_PAD], F32, kind="ExternalOutput")
    out_probs = nc.dram_tensor("out_probs", [128, N_CHUNKS], F32, kind="ExternalOutput")

    with tile.TileContext(nc) as tc:
        with (
            tc.tile_pool(name="const", bufs=1) as cpool,
            tc.tile_pool(name="hbuf", bufs=2) as hpool,
            tc.tile_pool(name="graw", bufs=8) as gpool,
            tc.tile_pool(name="pnt", bufs=1, space="PSUM") as pnt_pool,
            tc.tile_pool(name="ph", bufs=5, space="PSUM") as ph_pool,
            tc.tile_pool(name="plg", bufs=2, space="PSUM") as plg_pool,
            tc.tile_pool(name="dram", bufs=1, space="DRAM") as dpool,
        ):
            i0 = cpool.tile([128, C0 // 16], I16, tag="i0")
            nc.sync.dma_start(out=i0[:], in_=idx0[:])
            i1 = cpool.tile([128, C1 // 16], I16, tag="i1")
            nc.sync.dma_start(out=i1[:], in_=idx1[:])
            # ---- constant loads (bf16 weights via SWDGE cast-DMA) --------
            w1a = cpool.tile([128, H], BF16, tag="w1a")
            nc.gpsimd.dma_start(out=w1a[:], in_=w1[0:D, :])
            w2t = [cpool.tile([128, H], BF16, tag=f"w2_{k}", name=f"w2_{k}")
                   for k in range(2)]
            w3t = [cpool.tile([128, H], BF16, tag=f"w3_{k}", name=f"w3_{k}")
                   for k in range(2)]
            for k in range(2):
                nc.gpsimd.dma_start(out=w2t[k][:], in_=w2[k * 128:(k + 1) * 128, :])
                nc.gpsimd.dma_start(out=w3t[k][:], in_=w3[k * 128:(k + 1) * 128, :])
            w4s = cpool.tile([128, 2], BF16, tag="w4s")
            nc.gpsimd.dma_start(out=w4s[:], in_=packed[:, 6:8])

            w1b = cpool.tile([128, H], F32, tag="w1b")
            nc.sync.dma_start(out=w1b[:], in_=w1[D:2 * D, :])
            pk = cpool.tile([128, 131], F32, tag="pk")
            nc.sync.dma_start(out=pk[:], in_=packed[:])
            b1s = pk[:, 0:2]
            b2s = pk[:, 2:4]
            b3s = pk[:, 4:6]
            regTs = pk[:, 8:16]
            tails = pk[:, 16:26]
            masks = pk[:, 26:130]
            b4s = pk[0:1, 130:131]
            ohs = cpool.tile([N_REGIONS, A_PAD], BF16, tag="ohs")
            nc.sync.dma_start(out=ohs[:], in_=onehot[:])

            # ---- prologue: RP = region_emb @ W1b (fp32, [region, j]) -----
            rp_ps = plg_pool.tile([8, H], F32, space="PSUM", tag="lg")
            nc.tensor.matmul(out=rp_ps[:], lhsT=regTs, rhs=w1b[:],
                             start=True, stop=True)
            rps = cpool.tile([8, H], BF16, tag="rps")
            nc.vector.tensor_copy(out=rps[:], in_=rp_ps[:])

            # ---- prologue: c_tail = tail @ W1[256:] + b1 (fp32) ----------
            w1ta = cpool.tile([128, 9 * H], F32, tag="w1ta")
            nc.sync.dma_start(
                out=w1ta[:].rearrange("p (kt h) -> p kt h", kt=9),
                in_=w1[2 * D:2 * D + 9 * 128, :].rearrange(
                    "(kt p) h -> p kt h", p=128))
            w1t_last = cpool.tile([128, H], F32, tag="w1t_last")
            nc.sync.dma_start(out=w1t_last[0:TAIL_LEN - 9 * 128, :],
                              in_=w1[2 * D + 9 * 128:IN_DIM, :])
            w1tt = [w1ta[:, kt * H:(kt + 1) * H] for kt in range(9)] + [w1t_last]
            ct_ps = plg_pool.tile([128, 2], F32, space="PSUM", tag="lg")
            for j in range(2):
                for kt in range(TAIL_KT):
                    kk = min(128, TAIL_LEN - kt * 128)
                    wsl = w1tt[kt]
                    nc.tensor.matmul(
                        out=ct_ps[:, j:j + 1],
                        lhsT=wsl[0:kk, j * 128:(j + 1) * 128],
                        rhs=tails[0:kk, kt:kt + 1],
                        start=(kt == 0), stop=(kt == TAIL_KT - 1))
            b1cs = cpool.tile([128, 2], F32, tag="b1cs")
            nc.vector.tensor_add(out=b1cs[:], in0=ct_ps[:], in1=b1s)

            lrow = cpool.tile([1, A_PAD], F32, tag="lrow")

            # ---- node gather setup: nts_all[d, slot] = node_emb[id(slot), d]
            nts_all = cpool.tile([128, A_PAD], BF16, tag="nts_all")
            ident = cpool.tile([128, 128], BF16, tag="ident")
            make_identity(nc, ident[:])
            gather_plan = (
                [(0, off, n, 0) for off, n in _gather_chunks(C0)]
                + [(C0, off, n, 1) for off, n in _gather_chunks(C1)])

            def emit_gather(gi):
                zone, off, n, grp = gather_plan[gi]
                graw = gpool.tile([128, n // 128, D], BF16, tag="graw",
                                  name="graw")
                gsrc = node_emb[0:SPLIT, :] if grp == 0 \
                    else node_emb[SPLIT:N_NODES, :]
                itile = i0 if grp == 0 else i1
                nc.gpsimd.dma_gather(
                    out_ap=graw[:],
                    in_ap=gsrc,
                    idxs_ap=itile[:, off // 16:(off + n) // 16],
                    num_idxs=n, num_idxs_reg=n,
                    elem_size=D, transpose=False, single_packet=False,
                    queue_num=1 + gi % 3)
                nt_ps = pnt_pool.tile([128, GCHUNK], BF16, space="PSUM",
                                      tag="nt_ps", name="nt_ps")
                for c in range(n // 128):
                    nc.tensor.transpose(
                        out=nt_ps[:, c * 128:(c + 1) * 128],
                        in_=graw[:, c, :], identity=ident[:])
                s0 = zone + off
                if gi % 2 == 0:
                    nc.scalar.activation(
                        out=nts_all[:, s0:s0 + n], in_=nt_ps[:, 0:n],
                        func=mybir.ActivationFunctionType.Copy)
                else:
                    nc.vector.tensor_copy(out=nts_all[:, s0:s0 + n],
                                          in_=nt_ps[:, 0:n])
                return n

            def evict_relu(engine, dst, src, bias_ap):
                if engine == "act":
                    nc.scalar.activation(
                        out=dst, in_=src,
                        func=mybir.ActivationFunctionType.Relu, bias=bias_ap)
                else:
                    nc.vector.tensor_scalar(
                        out=dst, in0=src, scalar1=bias_ap, scalar2=0.0,
                        op0=mybir.AluOpType.add, op1=mybir.AluOpType.max)

            # ---- main loop: sweeps of up to 4 action tiles ----------------
            # Layer-major inside a sweep so consecutive matmuls share their
            # stationary operand (walrus ldw-opt then elides the reloads).
            SWEEP = 2
            t0s = list(range(0, N_AT, SWEEP))
            lgT = cpool.tile([128, N_CHUNKS], F32, tag="lgT")
            expt = cpool.tile([128, N_CHUNKS], F32, tag="expt")
            em = cpool.tile([128, N_CHUNKS], F32, tag="em")
            srow = cpool.tile([128, 1], F32, tag="srow")
            shift = cpool.tile([128, 1], F32, tag="shift")
            nc.gpsimd.memset(shift[:], EXP_SHIFT)
            HALF_A = (N_AT // 2) * ATILE        # actions in half 0 (tiles 0-12)
            HROW = HALF_A // N_CHUNKS           # lgT rows covered (64)

            def softmax_prep_half(h):
                r0, r1 = (0, HROW) if h == 0 else (HROW, 128)
                a0, a1 = r0 * N_CHUNKS, r1 * N_CHUNKS
                nc.sync.dma_start(out=out_logits[0:1, a0:a1],
                                  in_=lrow[0:1, a0:a1])
                nc.sync.dma_start(
                    out=lgT[r0:r1, :],
                    in_=out_logits[0:1, a0:a1].rearrange(
                        "o (p t) -> (o p) t", p=r1 - r0))
                nc.scalar.activation(out=expt[r0:r1, :], in_=lgT[r0:r1, :],
                                     func=mybir.ActivationFunctionType.Exp,
                                     bias=shift[r0:r1, :], scale=1.0)
                nc.vector.tensor_tensor(out=em[r0:r1, :], in0=expt[r0:r1, :],
                                        in1=pk[r0:r1, 26:130],
                                        op=mybir.AluOpType.mult)
                nc.vector.tensor_reduce(out=srow[r0:r1, :], in_=em[r0:r1, :],
                                        axis=mybir.AxisListType.X,
                                        op=mybir.AluOpType.add)

            gi_next = 0
            covered = 0
            # prime the gather pipeline two chunks deep
            while gi_next < len(gather_plan) and covered < 2 * GCHUNK:
                covered += emit_gather(gi_next)
                gi_next += 1
            for t0 in t0s:
                need = min(t0 + SWEEP, N_AT) * ATILE
                # emit gathers until this sweep's slots (+1 chunk lookahead)
                # are in flight, so the PE stream alternates transposes and
                # matmuls in data-arrival order
                while gi_next < len(gather_plan) and covered < need + GCHUNK:
                    covered += emit_gather(gi_next)
                    gi_next += 1
                tiles = list(range(t0, min(t0 + SWEEP, N_AT)))
                sls = [slice(t * ATILE, (t + 1) * ATILE) for t in tiles]
                nt = len(tiles)

                # layer 1
                h1 = [[hpool.tile([128, ATILE], BF16, tag=f"h1_{j}_{i}",
                                  name=f"h1_{j}_{i}")
                       for j in range(2)] for i in range(nt)]
                for j in range(2):
                    hps = [ph_pool.tile([128, ATILE], F32, space="PSUM",
                                        tag="hps", name="hps")
                           for _ in range(nt)]
                    for i in range(nt):
                        nc.tensor.matmul(out=hps[i][:],
                                         lhsT=w1a[:, j * 128:(j + 1) * 128],
                                         rhs=nts_all[:, sls[i]],
                                         start=True, stop=False)
                    for i in range(nt):
                        nc.tensor.matmul(out=hps[i][:],
                                         lhsT=rps[0:8, j * 128:(j + 1) * 128],
                                         rhs=ohs[0:8, sls[i]],
                                         start=False, stop=True)
                    for i in range(nt):
                        evict_relu("act" if (i + j) % 2 == 0 else "dve",
                                   h1[i][j][:], hps[i][:], b1cs[:, j:j + 1])

                # layers 2 and 3
                hin = h1
                for li, (wt, bs) in enumerate(((w2t, b2s), (w3t, b3s))):
                    hout = [[hpool.tile([128, ATILE], BF16,
                                        tag=f"h{li + 2}_{j}_{i}",
                                        name=f"h{li + 2}_{j}_{i}")
                             for j in range(2)] for i in range(nt)]
                    for j in range(2):
                        hps = [ph_pool.tile([128, ATILE], F32, space="PSUM",
                                            tag="hps", name="hps")
                               for _ in range(nt)]
                        for k in range(2):
                            for i in range(nt):
                                nc.tensor.matmul(
                                    out=hps[i][:],
                                    lhsT=wt[k][:, j * 128:(j + 1) * 128],
                                    rhs=hin[i][k][:],
                                    start=(k == 0), stop=(k == 1))
                        for i in range(nt):
                            evict_relu("act" if (i + j + li) % 2 == 0 else "dve",
                                       hout[i][j][:], hps[i][:], bs[:, j:j + 1])
                    hin = hout

                # layer 4: logits
                for i in range(nt):
                    lg = plg_pool.tile([1, ATILE], F32, space="PSUM", tag="lg")
                    for k in range(2):
                        nc.tensor.matmul(out=lg[:], lhsT=w4s[:, k:k + 1],
                                         rhs=hin[i][k][:],
                                         start=(k == 0), stop=(k == 1))
                    nc.scalar.activation(
                        out=lrow[0:1, sls[i]], in_=lg[:],
                        func=mybir.ActivationFunctionType.Identity,
                        bias=b4s)
                if tiles[-1] == N_AT // 2 - 1 or \
                        (tiles[0] <= N_AT // 2 - 1 < tiles[-1]):
                    softmax_prep_half(0)

            # ---- softmax: second half (first half emitted mid-loop) -----
            softmax_prep_half(1)
            from concourse import bass_isa
            sall = cpool.tile([128, 1], F32, tag="sall")
            nc.gpsimd.partition_all_reduce(out_ap=sall[:], in_ap=srow[:],
                                           channels=128,
                                           reduce_op=bass_isa.ReduceOp.add)
            s_sb = cpool.tile([1, 1], F32, tag="s_sb")
            nc.vector.tensor_copy(out=s_sb[:], in_=sall[0:1, :])

            cc_in = dpool.tile([1, 1], F32, name="cc_in")
            cc_out = dpool.tile([1, 1], F32, addr_space="Shared", name="cc_out")
            nc.gpsimd.dma_start(out=cc_in[:], in_=s_sb[:])
            nc.gpsimd.collective_compute(
                "AllReduce", mybir.AluOpType.add,
                replica_groups=[list(range(N_CORES))],
                ins=[cc_in.opt()], outs=[cc_out.opt()])
            sg = cpool.tile([1, 1], F32, tag="sg")
            nc.gpsimd.dma_start(out=sg[:], in_=cc_out[:])

            sgb = cpool.tile([128, 1], F32, tag="sgb")
            nc.gpsimd.partition_broadcast(out_ap=sgb[:], in_ap=sg[:])
            rb = cpool.tile([128, 1], F32, tag="rb")
            nc.vector.reciprocal(out=rb[:], in_=sgb[:])

            probs = cpool.tile([128, N_CHUNKS], F32, tag="probs")
            nc.vector.tensor_scalar_mul(out=probs[:], in0=em[:], scalar1=rb[:])
            nc.sync.dma_start(out=out_probs[:], in_=probs[:])

    nc.compile()
    return nc


_GRAPH_CACHE = {}


def _get_graph():
    if "g" not in _GRAPH_CACHE:
        _GRAPH_CACHE["g"] = build_graph()
    return _GRAPH_CACHE["g"]


def _wrap_idx(ix):
    """int16 index layout for dma_gather: [16, N/16] column-wrapped,
    replicated 8x down the partitions."""
    w = ix.reshape(-1, 16).T
    return np.ascontiguousarray(np.tile(w, (8, 1)))


def make_in_maps(node_embeddings, region_embeddings, global_context,
                 W1, b1, W2, b2, W3, b3, W4, b4,
                 action_nodes, action_regions):
    """Host-side sharding / marshalling. Returns (in_maps, per-core metas)."""
    W1 = np.ascontiguousarray(W1, dtype=np.float32)
    W2 = np.ascontiguousarray(W2, dtype=np.float32)
    W3 = np.ascontiguousarray(W3, dtype=np.float32)
    an = np.asarray(action_nodes).astype(np.int64)
    ar = np.asarray(action_regions).astype(np.int64)
    node_bf16 = np.ascontiguousarray(
        np.asarray(node_embeddings, np.float32).astype(ml_dtypes.bfloat16))

    tail = np.concatenate([
        np.asarray(region_embeddings, np.float32).reshape(-1),
        np.asarray(global_context, np.float32).reshape(-1)])
    tail_pad = np.zeros(TAIL_KT * 128, np.float32)
    tail_pad[:TAIL_LEN] = tail
    tailc = np.ascontiguousarray(tail_pad.reshape(TAIL_KT, 128).T)

    pk_base = np.zeros((128, 131), np.float32)
    pk_base[:, 0:2] = np.asarray(b1, np.float32).reshape(2, 128).T
    pk_base[:, 2:4] = np.asarray(b2, np.float32).reshape(2, 128).T
    pk_base[:, 4:6] = np.asarray(b3, np.float32).reshape(2, 128).T
    pk_base[:, 6:8] = np.asarray(W4, np.float32).reshape(2, 128).T
    pk_base[:, 8:16] = np.asarray(region_embeddings, np.float32).T
    pk_base[:, 16:26] = tailc
    pk_base[0, 130] = np.asarray(b4, np.float32).reshape(-1)[0]

    in_maps, metas = [], []
    for c in range(N_CORES):
        s = c * A_PC
        nodes = an[s:s + A_PC]
        regions = ar[s:s + A_PC]
        grp = (nodes >= SPLIT).astype(np.int8)
        order = np.argsort(grp, kind="stable")      # group0 first, stable
        c0 = int((grp == 0).sum())
        c1 = A_PC - c0
        if c0 > C0 or c1 > C1:
            raise RuntimeError(
                f"core {c}: group sizes {c0}/{c1} exceed capacities {C0}/{C1}")
        sn = nodes[order]
        sr = regions[order]

        ix0 = np.zeros(C0, np.int16)
        ix0[:c0] = sn[:c0].astype(np.int16)
        ix1 = np.zeros(C1, np.int16)
        ix1[:c1] = (sn[c0:] - SPLIT).astype(np.int16)

        slots = np.concatenate([np.arange(c0), C0 + np.arange(c1)])
        oh = np.zeros((N_REGIONS, A_PAD), ml_dtypes.bfloat16)
        oh[sr, slots] = 1.0
        mask = np.zeros(A_PAD, np.float32)
        mask[slots] = 1.0

        pkc = pk_base.copy()
        pkc[:, 26:130] = mask.reshape(128, N_CHUNKS)
        in_maps.append({
            "node_emb": node_bf16,
            "w1": W1, "w2": W2, "w3": W3,
            "packed": pkc,
            "idx0": _wrap_idx(ix0), "idx1": _wrap_idx(ix1),
            "onehot": oh,
        })
        metas.append((order, slots))
    return in_maps, metas


def kernel(**inputs):
    nc = _get_graph()
    in_maps, metas = make_in_maps(**inputs)
    res = bass_utils.run_bass_kernel_spmd(
        nc, in_maps, core_ids=list(range(N_CORES)))
    probs = np.empty(A_FULL, np.float32)
    logits = np.empty(A_FULL, np.float32)
    for c in range(N_CORES):
        order, slots = metas[c]
        out = res.results[c]
        lg = out["out_logits"].reshape(-1)[slots]
        pb = out["out_probs"].reshape(-1)[slots]
        logits[c * A_PC:(c + 1) * A_PC][order] = lg
        probs[c * A_PC:(c + 1) * A_PC][order] = pb
    return probs, logits



# revision 3
# speedup vs baseline: 1.2356x; 1.2356x over previous
"""Trainium2 Bass kernel for the Actor MLP scorer (gnn_message_passing), v2.

Computation (see reference):
    node_e  = node_embeddings[action_nodes]          # [A, 128] gather
    feats   = [node_e | region_embeddings[action_regions] | const_tail]   # [A, 1427]
    h1..h3  = relu MLP (256 wide), logits = h3 @ W4 + b4                  # [A]
    probs   = softmax(logits) over ALL actions

Strategy (8 NeuronCores, data-parallel over actions):
  - Shard A=100000 actions as 12500/core, sorted by node-id bucket
    (< 32768 vs >= 32768) so the node gather can use the int16-indexed
    DMA-gather ucode over two base-offset views of a bf16 table copy.
    transpose=True gather deposits embeddings directly in [dim, action]
    layout (no PE transposes, no PSUM staging).
  - Layer 1 decomposition: feats @ W1 = node_e @ W1[:128]
        + onehot(region) @ (region_embeddings @ W1[128:256])
        + (tail @ W1[256:] + b1)  [host-precomputed constant bias].
    All constant projections (RPS, b1c) are computed on host.
  - Activations stay transposed ([feature, action]); matmuls bf16 with
    fp32 PSUM; relu+bias evictions split across ScalarE/VectorE.
  - No collectives: each core writes its logits; the global softmax
    normalization (exp/sum/divide) happens on host during unsharding.
"""

import sys

for _p in ("/opt/trn_rl_repo",):
    if _p not in sys.path:
        sys.path.insert(0, _p)

import numpy as np
import ml_dtypes
from concourse import bass, bacc, mybir, tile
from concourse import bass_utils
from concourse.masks import make_identity


# ---------------------------------------------------------------- constants
N_CORES = 8
A_FULL = 100000
N_NODES = 50000
N_REGIONS = 8
D = 128
H = 256
G = 147
IN_DIM = 2 * D + N_REGIONS * D + G          # 1427
F32 = mybir.dt.float32
BF16 = mybir.dt.bfloat16
I16 = mybir.dt.int16

A_PC = A_FULL // N_CORES                    # 12500
SPLIT = 32768                               # int16 index range boundary
C0 = 8704                                   # capacity, node id < 32768
C1 = 4608                                   # capacity, node id >= 32768
A_PAD = C0 + C1                             # 13312 = 26*512
ATILE = 512
N_AT = A_PAD // ATILE                       # 26
GCHUNK = 1024                               # idxs per dma_gather call

USE_TGATHER = True                          # dma_gather transpose mode


def _gather_chunks(total):
    out, off = [], 0
    while off < total:
        n = min(GCHUNK, total - off)
        out.append((off, n))
        off += n
    return out


def build_graph():
    nc = bacc.Bacc("TRN2", target_bir_lowering=False, debug=False,
                   num_devices=N_CORES, num_swdge_queues=4)

    # ---- I/O --------------------------------------------------------------
    node_emb = nc.dram_tensor("node_emb", [N_NODES, D], BF16, kind="ExternalInput")
    wa = nc.dram_tensor("wa", [D, H], BF16, kind="ExternalInput")
    w2b = nc.dram_tensor("w2b", [H, H], BF16, kind="ExternalInput")
    w3b = nc.dram_tensor("w3b", [H, H], BF16, kind="ExternalInput")
    rps_w = nc.dram_tensor("rps_w", [N_REGIONS, H], BF16, kind="ExternalInput")
    w4b = nc.dram_tensor("w4b", [128, 2], BF16, kind="ExternalInput")
    # cols 0:2 b1c | 2:4 b2 | 4:6 b3 | [0,6] b4
    packed = nc.dram_tensor("packed", [128, 8], F32, kind="ExternalInput")
    idx0 = nc.dram_tensor("idx0", [128, C0 // 16], I16, kind="ExternalInput")
    idx1 = nc.dram_tensor("idx1", [128, C1 // 16], I16, kind="ExternalInput")
    onehot = nc.dram_tensor("onehot", [N_REGIONS, A_PAD], BF16, kind="ExternalInput")

    out_logits = nc.dram_tensor("out_logits", [1, A_PAD], F32, kind="ExternalOutput")

    with tile.TileContext(nc) as tc:
        with (
            tc.tile_pool(name="const", bufs=1) as cpool,
            tc.tile_pool(name="hbuf", bufs=2) as hpool,
            tc.tile_pool(name="graw", bufs=8) as gpool,
            tc.tile_pool(name="pnt", bufs=1, space="PSUM") as pnt_pool,
            tc.tile_pool(name="ph", bufs=5, space="PSUM") as ph_pool,
            tc.tile_pool(name="plg", bufs=2, space="PSUM") as plg_pool,
        ):
            # ---- index loads first: gathers depend on them ---------------
            i0 = cpool.tile([128, C0 // 16], I16, tag="i0")
            nc.sync.dma_start(out=i0[:], in_=idx0[:])
            i1 = cpool.tile([128, C1 // 16], I16, tag="i1")
            nc.sync.dma_start(out=i1[:], in_=idx1[:])

            # ---- constant loads (host pre-cast bf16) ----------------------
            w1a = cpool.tile([128, H], BF16, tag="w1a")
            nc.sync.dma_start(out=w1a[:], in_=wa[:])
            rps = cpool.tile([N_REGIONS, H], BF16, tag="rps")
            nc.sync.dma_start(out=rps[:], in_=rps_w[:])
            pk = cpool.tile([128, 8], F32, tag="pk")
            nc.sync.dma_start(out=pk[:], in_=packed[:])
            ohs = cpool.tile([N_REGIONS, A_PAD], BF16, tag="ohs")
            nc.scalar.dma_start(out=ohs[:], in_=onehot[:])
            w2t = [cpool.tile([128, H], BF16, tag=f"w2_{k}", name=f"w2_{k}")
                   for k in range(2)]
            w3t = [cpool.tile([128, H], BF16, tag=f"w3_{k}", name=f"w3_{k}")
                   for k in range(2)]
            for k in range(2):
                nc.scalar.dma_start(out=w2t[k][:], in_=w2b[k * 128:(k + 1) * 128, :])
                nc.scalar.dma_start(out=w3t[k][:], in_=w3b[k * 128:(k + 1) * 128, :])
            w4s = cpool.tile([128, 2], BF16, tag="w4s")
            nc.sync.dma_start(out=w4s[:], in_=w4b[:])

            b1s = pk[:, 0:2]
            b2s = pk[:, 2:4]
            b3s = pk[:, 4:6]
            b4s = pk[0:1, 6:7]

            lrow = cpool.tile([1, A_PAD], F32, tag="lrow")

            # ---- node gather: nts_all[d, slot] = node_emb[id(slot), d] ---
            nts_all = cpool.tile([128, A_PAD], BF16, tag="nts_all")
            gather_plan = (
                [(0, off, n, 0) for off, n in _gather_chunks(C0)]
                + [(C0, off, n, 1) for off, n in _gather_chunks(C1)])

            if not USE_TGATHER:
                ident = cpool.tile([128, 128], BF16, tag="ident")
                make_identity(nc, ident[:])

            def emit_gather(gi):
                zone, off, n, grp = gather_plan[gi]
                gsrc = node_emb[0:SPLIT, :] if grp == 0 \
                    else node_emb[SPLIT:N_NODES, :]
                itile = i0 if grp == 0 else i1
                s0 = zone + off
                if USE_TGATHER:
                    nc.gpsimd.dma_gather(
                        out_ap=nts_all[:, s0:s0 + n].unsqueeze(1),
                        in_ap=gsrc,
                        idxs_ap=itile[:, off // 16:(off + n) // 16],
                        num_idxs=n, num_idxs_reg=n,
                        elem_size=D, transpose=True, single_packet=False,
                        queue_num=1)
                    return n
                graw = gpool.tile([128, n // 128, D], BF16, tag="graw",
                                  name="graw")
                nc.gpsimd.dma_gather(
                    out_ap=graw[:],
                    in_ap=gsrc,
                    idxs_ap=itile[:, off // 16:(off + n) // 16],
                    num_idxs=n, num_idxs_reg=n,
                    elem_size=D, transpose=False, single_packet=False,
                    queue_num=1 + gi % 3)
                nt_ps = pnt_pool.tile([128, GCHUNK], BF16, space="PSUM",
                                      tag="nt_ps", name="nt_ps")
                for c in range(n // 128):
                    nc.tensor.transpose(
                        out=nt_ps[:, c * 128:(c + 1) * 128],
                        in_=graw[:, c, :], identity=ident[:])
                if gi % 2 == 0:
                    nc.scalar.activation(
                        out=nts_all[:, s0:s0 + n], in_=nt_ps[:, 0:n],
                        func=mybir.ActivationFunctionType.Copy)
                else:
                    nc.vector.tensor_copy(out=nts_all[:, s0:s0 + n],
                                          in_=nt_ps[:, 0:n])
                return n

            def evict_relu(engine, dst, src, bias_ap):
                if engine == "act":
                    nc.scalar.activation(
                        out=dst, in_=src,
                        func=mybir.ActivationFunctionType.Relu, bias=bias_ap)
                else:
                    nc.vector.tensor_scalar(
                        out=dst, in0=src, scalar1=bias_ap, scalar2=0.0,
                        op0=mybir.AluOpType.add, op1=mybir.AluOpType.max)

            # ---- main loop: sweeps of 2 action tiles ----------------------
            SWEEP = 2
            t0s = list(range(0, N_AT, SWEEP))
            out_done = 0                       # cols already DMAed out

            def flush_logits(upto):
                nonlocal out_done
                if upto > out_done:
                    nc.sync.dma_start(out=out_logits[0:1, out_done:upto],
                                      in_=lrow[0:1, out_done:upto])
                    out_done = upto

            gi_next = 0
            covered = 0
            # prime the gather pipeline two chunks deep
            while gi_next < len(gather_plan) and covered < 2 * GCHUNK:
                covered += emit_gather(gi_next)
                gi_next += 1
            for si, t0 in enumerate(t0s):
                need = min(t0 + SWEEP, N_AT) * ATILE
                while gi_next < len(gather_plan) and covered < need + GCHUNK:
                    covered += emit_gather(gi_next)
                    gi_next += 1
                tiles = list(range(t0, min(t0 + SWEEP, N_AT)))
                sls = [slice(t * ATILE, (t + 1) * ATILE) for t in tiles]
                nt = len(tiles)

                # layer 1
                h1 = [[hpool.tile([128, ATILE], BF16, tag=f"h1_{j}_{i}",
                                  name=f"h1_{j}_{i}")
                       for j in range(2)] for i in range(nt)]
                for j in range(2):
                    hps = [ph_pool.tile([128, ATILE], F32, space="PSUM",
                                        tag="hps", name="hps")
                           for _ in range(nt)]
                    for i in range(nt):
                        nc.tensor.matmul(out=hps[i][:],
                                         lhsT=w1a[:, j * 128:(j + 1) * 128],
                                         rhs=nts_all[:, sls[i]],
                                         start=True, stop=False)
                    for i in range(nt):
                        nc.tensor.matmul(out=hps[i][:],
                                         lhsT=rps[0:8, j * 128:(j + 1) * 128],
                                         rhs=ohs[0:8, sls[i]],
                                         start=False, stop=True)
                    for i in range(nt):
                        evict_relu("act" if (i + j) % 2 == 0 else "dve",
                                   h1[i][j][:], hps[i][:], b1s[:, j:j + 1])

                # layers 2 and 3
                hin = h1
                for li, (wt, bs) in enumerate(((w2t, b2s), (w3t, b3s))):
                    hout = [[hpool.tile([128, ATILE], BF16,
                                        tag=f"h{li + 2}_{j}_{i}",
                                        name=f"h{li + 2}_{j}_{i}")
                             for j in range(2)] for i in range(nt)]
                    for j in range(2):
                        hps = [ph_pool.tile([128, ATILE], F32, space="PSUM",
                                            tag="hps", name="hps")
                               for _ in range(nt)]
                        for k in range(2):
                            for i in range(nt):
                                nc.tensor.matmul(
                                    out=hps[i][:],
                                    lhsT=wt[k][:, j * 128:(j + 1) * 128],
                                    rhs=hin[i][k][:],
                                    start=(k == 0), stop=(k == 1))
                        for i in range(nt):
                            evict_relu("act" if (i + j + li) % 2 == 0 else "dve",
                                       hout[i][j][:], hps[i][:], bs[:, j:j + 1])
                    hin = hout

                # layer 4: logits
                for i in range(nt):
                    lg = plg_pool.tile([1, ATILE], F32, space="PSUM", tag="lg")
                    for k in range(2):
                        nc.tensor.matmul(out=lg[:], lhsT=w4s[:, k:k + 1],
                                         rhs=hin[i][k][:],
                                         start=(k == 0), stop=(k == 1))
                    nc.scalar.activation(
                        out=lrow[0:1, sls[i]], in_=lg[:],
                        func=mybir.ActivationFunctionType.Identity,
                        bias=b4s)
                if si in (3, 6, 9):
                    flush_logits((t0 + SWEEP) * ATILE)

            flush_logits(A_PAD)

    nc.compile()
    return nc


_GRAPH_CACHE = {}


def _get_graph():
    if "g" not in _GRAPH_CACHE:
        _GRAPH_CACHE["g"] = build_graph()
    return _GRAPH_CACHE["g"]


def _wrap_idx(ix):
    """int16 index layout for dma_gather: [16, N/16] column-wrapped,
    replicated 8x down the partitions."""
    w = ix.reshape(-1, 16).T
    return np.ascontiguousarray(np.tile(w, (8, 1)))


def make_in_maps(node_embeddings, region_embeddings, global_context,
                 W1, b1, W2, b2, W3, b3, W4, b4,
                 action_nodes, action_regions):
    """Host-side sharding / marshalling. Returns (in_maps, per-core metas)."""
    W1 = np.asarray(W1, np.float32)
    an = np.asarray(action_nodes).astype(np.int64)
    ar = np.asarray(action_regions).astype(np.int64)
    node_bf16 = np.ascontiguousarray(
        np.asarray(node_embeddings, np.float32).astype(ml_dtypes.bfloat16))
    region_embeddings = np.asarray(region_embeddings, np.float32)

    tail = np.concatenate([
        region_embeddings.reshape(-1),
        np.asarray(global_context, np.float32).reshape(-1)])
    b1c = (np.asarray(b1, np.float32)
           + tail @ W1[2 * D:IN_DIM, :]).astype(np.float32)   # [256]
    rps_np = np.ascontiguousarray(
        (region_embeddings @ W1[D:2 * D, :]).astype(ml_dtypes.bfloat16))
    wa_np = np.ascontiguousarray(W1[0:D, :].astype(ml_dtypes.bfloat16))
    w2b_np = np.ascontiguousarray(
        np.asarray(W2, np.float32).astype(ml_dtypes.bfloat16))
    w3b_np = np.ascontiguousarray(
        np.asarray(W3, np.float32).astype(ml_dtypes.bfloat16))
    w4b_np = np.ascontiguousarray(
        np.asarray(W4, np.float32).reshape(2, 128).T.astype(ml_dtypes.bfloat16))

    pk_base = np.zeros((128, 8), np.float32)
    pk_base[:, 0:2] = b1c.reshape(2, 128).T
    pk_base[:, 2:4] = np.asarray(b2, np.float32).reshape(2, 128).T
    pk_base[:, 4:6] = np.asarray(b3, np.float32).reshape(2, 128).T
    pk_base[0, 6] = np.asarray(b4, np.float32).reshape(-1)[0]

    in_maps, metas = [], []
    for c in range(N_CORES):
        s = c * A_PC
        nodes = an[s:s + A_PC]
        regions = ar[s:s + A_PC]
        grp = (nodes >= SPLIT).astype(np.int8)
        order = np.argsort(grp, kind="stable")      # group0 first, stable
        c0 = int((grp == 0).sum())
        c1 = A_PC - c0
        if c0 > C0 or c1 > C1:
            raise RuntimeError(
                f"core {c}: group sizes {c0}/{c1} exceed capacities {C0}/{C1}")
        sn = nodes[order]
        sr = regions[order]

        ix0 = np.zeros(C0, np.int16)
        ix0[:c0] = sn[:c0].astype(np.int16)
        ix1 = np.zeros(C1, np.int16)
        ix1[:c1] = (sn[c0:] - SPLIT).astype(np.int16)

        slots = np.concatenate([np.arange(c0), C0 + np.arange(c1)])
        oh = np.zeros((N_REGIONS, A_PAD), ml_dtypes.bfloat16)
        oh[sr, slots] = 1.0

        in_maps.append({
            "node_emb": node_bf16,
            "wa": wa_np, "w2b": w2b_np, "w3b": w3b_np,
            "rps_w": rps_np, "w4b": w4b_np,
            "packed": pk_base,
            "idx0": _wrap_idx(ix0), "idx1": _wrap_idx(ix1),
            "onehot": oh,
        })
        metas.append((order, slots))
    return in_maps, metas


def _unshard(results, metas):
    logits = np.empty(A_FULL, np.float32)
    for c in range(N_CORES):
        order, slots = metas[c]
        lg = np.asarray(results[c]).reshape(-1)[slots]
        logits[c * A_PC:(c + 1) * A_PC][order] = lg
    le = logits.astype(np.float64)
    e = np.exp(le - le.max())
    probs = (e / e.sum()).astype(np.float32)
    return probs, logits


def kernel(**inputs):
    nc = _get_graph()
    in_maps, metas = make_in_maps(**inputs)
    res = bass_utils.run_bass_kernel_spmd(
        nc, in_maps, core_ids=list(range(N_CORES)))
    return _unshard([res.results[c]["out_logits"] for c in range(N_CORES)],
                    metas)


# revision 4
# speedup vs baseline: 1.5575x; 1.2605x over previous
"""Trainium2 Bass kernel for the Actor MLP scorer (gnn_message_passing), v2.

Computation (see reference):
    node_e  = node_embeddings[action_nodes]          # [A, 128] gather
    feats   = [node_e | region_embeddings[action_regions] | const_tail]   # [A, 1427]
    h1..h3  = relu MLP (256 wide), logits = h3 @ W4 + b4                  # [A]
    probs   = softmax(logits) over ALL actions

Strategy (8 NeuronCores, data-parallel over actions):
  - Shard A=100000 actions as 12500/core, sorted by node-id bucket
    (< 32768 vs >= 32768) so the node gather can use the int16-indexed
    DMA-gather ucode over two base-offset views of a bf16 table copy.
    transpose=True gather deposits embeddings directly in [dim, action]
    layout (no PE transposes, no PSUM staging).
  - Layer 1 decomposition: feats @ W1 = node_e @ W1[:128]
        + onehot(region) @ (region_embeddings @ W1[128:256])
        + (tail @ W1[256:] + b1)  [host-precomputed constant bias].
    All constant projections (RPS, b1c) are computed on host.
  - Activations stay transposed ([feature, action]); matmuls bf16 with
    fp32 PSUM; relu+bias evictions split across ScalarE/VectorE.
  - No collectives: each core writes its logits; the global softmax
    normalization (exp/sum/divide) happens on host during unsharding.
"""

import sys

for _p in ("/opt/trn_rl_repo",):
    if _p not in sys.path:
        sys.path.insert(0, _p)

import numpy as np
import ml_dtypes
from concourse import bass, bacc, mybir, tile
from concourse import bass_utils
from concourse.masks import make_identity


# ---------------------------------------------------------------- constants
N_CORES = 8
A_FULL = 100000
N_NODES = 50000
N_REGIONS = 8
D = 128
H = 256
G = 147
IN_DIM = 2 * D + N_REGIONS * D + G          # 1427
F32 = mybir.dt.float32
BF16 = mybir.dt.bfloat16
I16 = mybir.dt.int16

A_PC = A_FULL // N_CORES                    # 12500
SPLIT = 32768                               # int16 index range boundary
C0 = 8704                                   # capacity, node id < 32768
C1 = 4608                                   # capacity, node id >= 32768
A_PAD = C0 + C1                             # 13312 = 26*512
ATILE = 512
N_AT = A_PAD // ATILE                       # 26
GCHUNK = 1024                               # idxs per dma_gather call

USE_TGATHER = False                         # dma_gather transpose mode


def _gather_chunks(total):
    out, off = [], 0
    while off < total:
        n = min(GCHUNK, total - off)
        out.append((off, n))
        off += n
    return out


def build_graph():
    nc = bacc.Bacc("TRN2", target_bir_lowering=False, debug=False,
                   num_devices=N_CORES, num_swdge_queues=4)

    # ---- I/O --------------------------------------------------------------
    node_emb = nc.dram_tensor("node_emb", [N_NODES, D], BF16, kind="ExternalInput")
    wa = nc.dram_tensor("wa", [D, H], BF16, kind="ExternalInput")
    w2b = nc.dram_tensor("w2b", [H, H], BF16, kind="ExternalInput")
    w3b = nc.dram_tensor("w3b", [H, H], BF16, kind="ExternalInput")
    rps_w = nc.dram_tensor("rps_w", [N_REGIONS, H], BF16, kind="ExternalInput")
    w4b = nc.dram_tensor("w4b", [128, 2], BF16, kind="ExternalInput")
    # cols 0:2 b1c | 2:4 b2 | 4:6 b3 | [0,6] b4
    packed = nc.dram_tensor("packed", [128, 8], F32, kind="ExternalInput")
    idx0 = nc.dram_tensor("idx0", [128, C0 // 16], I16, kind="ExternalInput")
    idx1 = nc.dram_tensor("idx1", [128, C1 // 16], I16, kind="ExternalInput")
    onehot = nc.dram_tensor("onehot", [N_REGIONS, A_PAD], BF16, kind="ExternalInput")

    out_logits = nc.dram_tensor("out_logits", [1, A_PAD], F32, kind="ExternalOutput")

    with tile.TileContext(nc) as tc:
        with (
            tc.tile_pool(name="const", bufs=1) as cpool,
            tc.tile_pool(name="hbuf", bufs=2) as hpool,
            tc.tile_pool(name="graw", bufs=8) as gpool,
            tc.tile_pool(name="pnt", bufs=1, space="PSUM") as pnt_pool,
            tc.tile_pool(name="ph", bufs=5, space="PSUM") as ph_pool,
            tc.tile_pool(name="plg", bufs=2, space="PSUM") as plg_pool,
        ):
            # ---- index loads first: gathers depend on them ---------------
            i0 = cpool.tile([128, C0 // 16], I16, tag="i0")
            nc.sync.dma_start(out=i0[:], in_=idx0[:])
            i1 = cpool.tile([128, C1 // 16], I16, tag="i1")
            nc.sync.dma_start(out=i1[:], in_=idx1[:])

            # ---- constant loads (host pre-cast bf16) ----------------------
            w1a = cpool.tile([128, H], BF16, tag="w1a")
            nc.sync.dma_start(out=w1a[:], in_=wa[:])
            rps = cpool.tile([N_REGIONS, H], BF16, tag="rps")
            nc.sync.dma_start(out=rps[:], in_=rps_w[:])
            pk = cpool.tile([128, 8], F32, tag="pk")
            nc.sync.dma_start(out=pk[:], in_=packed[:])
            ohs = cpool.tile([N_REGIONS, A_PAD], BF16, tag="ohs")
            nc.scalar.dma_start(out=ohs[:], in_=onehot[:])
            w2t = [cpool.tile([128, H], BF16, tag=f"w2_{k}", name=f"w2_{k}")
                   for k in range(2)]
            w3t = [cpool.tile([128, H], BF16, tag=f"w3_{k}", name=f"w3_{k}")
                   for k in range(2)]
            for k in range(2):
                nc.scalar.dma_start(out=w2t[k][:], in_=w2b[k * 128:(k + 1) * 128, :])
                nc.scalar.dma_start(out=w3t[k][:], in_=w3b[k * 128:(k + 1) * 128, :])
            w4s = cpool.tile([128, 2], BF16, tag="w4s")
            nc.sync.dma_start(out=w4s[:], in_=w4b[:])

            b1s = pk[:, 0:2]
            b2s = pk[:, 2:4]
            b3s = pk[:, 4:6]
            b4s = pk[0:1, 6:7]

            lrow = cpool.tile([1, A_PAD], F32, tag="lrow")

            # ---- node gather: nts_all[d, slot] = node_emb[id(slot), d] ---
            nts_all = cpool.tile([128, A_PAD], BF16, tag="nts_all")
            gather_plan = (
                [(0, off, n, 0) for off, n in _gather_chunks(C0)]
                + [(C0, off, n, 1) for off, n in _gather_chunks(C1)])

            if not USE_TGATHER:
                ident = cpool.tile([128, 128], BF16, tag="ident")
                make_identity(nc, ident[:])

            def emit_gather(gi):
                zone, off, n, grp = gather_plan[gi]
                gsrc = node_emb[0:SPLIT, :] if grp == 0 \
                    else node_emb[SPLIT:N_NODES, :]
                itile = i0 if grp == 0 else i1
                s0 = zone + off
                if USE_TGATHER:
                    nc.gpsimd.dma_gather(
                        out_ap=nts_all[:, s0:s0 + n].unsqueeze(1),
                        in_ap=gsrc,
                        idxs_ap=itile[:, off // 16:(off + n) // 16],
                        num_idxs=n, num_idxs_reg=n,
                        elem_size=D, transpose=True, single_packet=False,
                        queue_num=1)
                    return n
                graw = gpool.tile([128, n // 128, D], BF16, tag="graw",
                                  name="graw")
                nc.gpsimd.dma_gather(
                    out_ap=graw[:],
                    in_ap=gsrc,
                    idxs_ap=itile[:, off // 16:(off + n) // 16],
                    num_idxs=n, num_idxs_reg=n,
                    elem_size=D, transpose=False, single_packet=False,
                    queue_num=1 + gi % 3)
                nt_ps = pnt_pool.tile([128, GCHUNK], BF16, space="PSUM",
                                      tag="nt_ps", name="nt_ps")
                for c in range(n // 128):
                    nc.tensor.transpose(
                        out=nt_ps[:, c * 128:(c + 1) * 128],
                        in_=graw[:, c, :], identity=ident[:])
                if gi % 2 == 0:
                    nc.scalar.activation(
                        out=nts_all[:, s0:s0 + n], in_=nt_ps[:, 0:n],
                        func=mybir.ActivationFunctionType.Copy)
                else:
                    nc.vector.tensor_copy(out=nts_all[:, s0:s0 + n],
                                          in_=nt_ps[:, 0:n])
                return n

            def evict_relu(engine, dst, src, bias_ap):
                if engine == "act":
                    nc.scalar.activation(
                        out=dst, in_=src,
                        func=mybir.ActivationFunctionType.Relu, bias=bias_ap)
                else:
                    nc.vector.tensor_scalar(
                        out=dst, in0=src, scalar1=bias_ap, scalar2=0.0,
                        op0=mybir.AluOpType.add, op1=mybir.AluOpType.max)

            # ---- main loop: sweeps of 2 action tiles ----------------------
            SWEEP = 2
            t0s = list(range(0, N_AT, SWEEP))
            out_done = 0                       # cols already DMAed out

            def flush_logits(upto):
                nonlocal out_done
                if upto > out_done:
                    nc.sync.dma_start(out=out_logits[0:1, out_done:upto],
                                      in_=lrow[0:1, out_done:upto])
                    out_done = upto

            gi_next = 0
            covered = 0
            # prime the gather pipeline two chunks deep
            while gi_next < len(gather_plan) and covered < 2 * GCHUNK:
                covered += emit_gather(gi_next)
                gi_next += 1
            for si, t0 in enumerate(t0s):
                need = min(t0 + SWEEP, N_AT) * ATILE
                while gi_next < len(gather_plan) and covered < need + GCHUNK:
                    covered += emit_gather(gi_next)
                    gi_next += 1
                tiles = list(range(t0, min(t0 + SWEEP, N_AT)))
                sls = [slice(t * ATILE, (t + 1) * ATILE) for t in tiles]
                nt = len(tiles)

                # layer 1
                h1 = [[hpool.tile([128, ATILE], BF16, tag=f"h1_{j}_{i}",
                                  name=f"h1_{j}_{i}")
                       for j in range(2)] for i in range(nt)]
                for j in range(2):
                    hps = [ph_pool.tile([128, ATILE], F32, space="PSUM",
                                        tag="hps", name="hps")
                           for _ in range(nt)]
                    for i in range(nt):
                        nc.tensor.matmul(out=hps[i][:],
                                         lhsT=w1a[:, j * 128:(j + 1) * 128],
                                         rhs=nts_all[:, sls[i]],
                                         start=True, stop=False)
                    for i in range(nt):
                        nc.tensor.matmul(out=hps[i][:],
                                         lhsT=rps[0:8, j * 128:(j + 1) * 128],
                                         rhs=ohs[0:8, sls[i]],
                                         start=False, stop=True)
                    for i in range(nt):
                        evict_relu("act" if (i + j) % 2 == 0 else "dve",
                                   h1[i][j][:], hps[i][:], b1s[:, j:j + 1])

                # layers 2 and 3
                hin = h1
                for li, (wt, bs) in enumerate(((w2t, b2s), (w3t, b3s))):
                    hout = [[hpool.tile([128, ATILE], BF16,
                                        tag=f"h{li + 2}_{j}_{i}",
                                        name=f"h{li + 2}_{j}_{i}")
                             for j in range(2)] for i in range(nt)]
                    for j in range(2):
                        hps = [ph_pool.tile([128, ATILE], F32, space="PSUM",
                                            tag="hps", name="hps")
                               for _ in range(nt)]
                        for k in range(2):
                            for i in range(nt):
                                nc.tensor.matmul(
                                    out=hps[i][:],
                                    lhsT=wt[k][:, j * 128:(j + 1) * 128],
                                    rhs=hin[i][k][:],
                                    start=(k == 0), stop=(k == 1))
                        for i in range(nt):
                            evict_relu("act" if (i + j + li) % 2 == 0 else "dve",
                                       hout[i][j][:], hps[i][:], bs[:, j:j + 1])
                    hin = hout

                # layer 4: logits
                for i in range(nt):
                    lg = plg_pool.tile([1, ATILE], F32, space="PSUM", tag="lg")
                    for k in range(2):
                        nc.tensor.matmul(out=lg[:], lhsT=w4s[:, k:k + 1],
                                         rhs=hin[i][k][:],
                                         start=(k == 0), stop=(k == 1))
                    nc.scalar.activation(
                        out=lrow[0:1, sls[i]], in_=lg[:],
                        func=mybir.ActivationFunctionType.Identity,
                        bias=b4s)
                if si in (3, 6, 9):
                    flush_logits((t0 + SWEEP) * ATILE)

            flush_logits(A_PAD)

    nc.compile()
    return nc


_GRAPH_CACHE = {}


def _get_graph():
    if "g" not in _GRAPH_CACHE:
        _GRAPH_CACHE["g"] = build_graph()
    return _GRAPH_CACHE["g"]


def _wrap_idx(ix):
    """int16 index layout for dma_gather: [16, N/16] column-wrapped,
    replicated 8x down the partitions."""
    w = ix.reshape(-1, 16).T
    return np.ascontiguousarray(np.tile(w, (8, 1)))


def make_in_maps(node_embeddings, region_embeddings, global_context,
                 W1, b1, W2, b2, W3, b3, W4, b4,
                 action_nodes, action_regions):
    """Host-side sharding / marshalling. Returns (in_maps, per-core metas)."""
    W1 = np.asarray(W1, np.float32)
    an = np.asarray(action_nodes).astype(np.int64)
    ar = np.asarray(action_regions).astype(np.int64)
    node_bf16 = np.ascontiguousarray(
        np.asarray(node_embeddings, np.float32).astype(ml_dtypes.bfloat16))
    region_embeddings = np.asarray(region_embeddings, np.float32)

    tail = np.concatenate([
        region_embeddings.reshape(-1),
        np.asarray(global_context, np.float32).reshape(-1)])
    b1c = (np.asarray(b1, np.float32)
           + tail @ W1[2 * D:IN_DIM, :]).astype(np.float32)   # [256]
    rps_np = np.ascontiguousarray(
        (region_embeddings @ W1[D:2 * D, :]).astype(ml_dtypes.bfloat16))
    wa_np = np.ascontiguousarray(W1[0:D, :].astype(ml_dtypes.bfloat16))
    w2b_np = np.ascontiguousarray(
        np.asarray(W2, np.float32).astype(ml_dtypes.bfloat16))
    w3b_np = np.ascontiguousarray(
        np.asarray(W3, np.float32).astype(ml_dtypes.bfloat16))
    w4b_np = np.ascontiguousarray(
        np.asarray(W4, np.float32).reshape(2, 128).T.astype(ml_dtypes.bfloat16))

    pk_base = np.zeros((128, 8), np.float32)
    pk_base[:, 0:2] = b1c.reshape(2, 128).T
    pk_base[:, 2:4] = np.asarray(b2, np.float32).reshape(2, 128).T
    pk_base[:, 4:6] = np.asarray(b3, np.float32).reshape(2, 128).T
    pk_base[0, 6] = np.asarray(b4, np.float32).reshape(-1)[0]

    in_maps, metas = [], []
    for c in range(N_CORES):
        s = c * A_PC
        nodes = an[s:s + A_PC]
        regions = ar[s:s + A_PC]
        grp = (nodes >= SPLIT).astype(np.int8)
        order = np.argsort(grp, kind="stable")      # group0 first, stable
        c0 = int((grp == 0).sum())
        c1 = A_PC - c0
        if c0 > C0 or c1 > C1:
            raise RuntimeError(
                f"core {c}: group sizes {c0}/{c1} exceed capacities {C0}/{C1}")
        sn = nodes[order]
        sr = regions[order]

        ix0 = np.zeros(C0, np.int16)
        ix0[:c0] = sn[:c0].astype(np.int16)
        ix1 = np.zeros(C1, np.int16)
        ix1[:c1] = (sn[c0:] - SPLIT).astype(np.int16)

        slots = np.concatenate([np.arange(c0), C0 + np.arange(c1)])
        oh = np.zeros((N_REGIONS, A_PAD), ml_dtypes.bfloat16)
        oh[sr, slots] = 1.0

        in_maps.append({
            "node_emb": node_bf16,
            "wa": wa_np, "w2b": w2b_np, "w3b": w3b_np,
            "rps_w": rps_np, "w4b": w4b_np,
            "packed": pk_base,
            "idx0": _wrap_idx(ix0), "idx1": _wrap_idx(ix1),
            "onehot": oh,
        })
        metas.append((order, slots))
    return in_maps, metas


def _unshard(results, metas):
    logits = np.empty(A_FULL, np.float32)
    for c in range(N_CORES):
        order, slots = metas[c]
        lg = np.asarray(results[c]).reshape(-1)[slots]
        logits[c * A_PC:(c + 1) * A_PC][order] = lg
    le = logits.astype(np.float64)
    e = np.exp(le - le.max())
    probs = (e / e.sum()).astype(np.float32)
    return probs, logits


def kernel(**inputs):
    nc = _get_graph()
    in_maps, metas = make_in_maps(**inputs)
    res = bass_utils.run_bass_kernel_spmd(
        nc, in_maps, core_ids=list(range(N_CORES)))
    return _unshard([res.results[c]["out_logits"] for c in range(N_CORES)],
                    metas)
